# revision 12
# baseline (speedup 1.0000x reference)
"""Stride-2 bilinear upsampling (block-diagonal conv_transpose2d) on 8 NeuronCores.

v2 design, driven by measured DVE perf modes (fp32 tensor ops and ALL
scalar_tensor_tensor variants run 1x; bf16 tensor_tensor has the 2x uop,
bf16 tensor_scalar the 4x uop; ACT is ~1 elem/cycle any dtype; SWDGE DMA
casts bf16->f32 at line rate):

  S = a^2 * X          (ACT, f32->bf16; a = outer tap, filter f = [a,3a,3a,a])
  T = 3 * S            (DVE tensor_scalar, 4x)
  U[2m], U[2m+1] = T[m] + S[m+1], S[m] + T[m+1]     (DVE tt 2x, W-pass)
  V = 3 * U            (ACT, bf16->bf16)
  Z[2m], Z[2m+1] = V[m] + U[m+1], U[m] + V[m+1]     (DVE tt 2x, H-pass)
  out = f32(Z)         (SWDGE cast-DMA bf16->f32, GpSimd ring)

U is 0.25x the W-upsampled image, so every blend is a plain add of
pre-scaled operands.  Ghost rows/cols (zeros) absorb all edge handling.
Channel-parallel: 32 ch x 4 batch = 128 images/core, one per partition.
"""

import numpy as np

N, C, H, W = 4, 256, 128, 128
OH, OW = 258, 258
NCORES = 8
CPC = C // NCORES          # 32 channels per core
NIMG = N * CPC             # 128 images per core (one per SBUF partition)

_CACHE = {}


def _legalize_waits(nc, mybir):
    """Split multi-wait sync_info into standalone single-wait EventSemaphore
    instructions (this build encodes at most one sync-wait per instruction)."""
    n = 0
    for func in nc.m.functions:
        for block in func.blocks:
            out = []
            for inst in block.instructions:
                si = inst.sync_info
                if si is not None and si.on_wait is not None and len(si.on_wait) > 1:
                    waits = list(si.on_wait)
                    for k, w in enumerate(waits[:-1]):
                        out.append(mybir.InstEventSemaphore(
                            name=f"{inst.name}-hw{k}",
                            opcode="EventSemaphore",
                            engine=inst.engine,
                            ins=[], outs=[],
                            sync_info=mybir.SyncInfo(on_wait=[w], on_update=[]),
                        ))
                        n += 1
                    inst.sync_info = mybir.SyncInfo(
                        on_wait=[waits[-1]], on_update=list(si.on_update))
                out.append(inst)
            block.instructions = out
    return n


def _build_bass(scale, strips=None, bufs_x=6, bufs_s=2, bufs_v=2, bufs_z=3,
                in_ring="sync", out_mode="bf16_swdge",
                split_last=1, hoist_in=True, v_dve_edge=4):
    """Per-core view: x[128,128,128]f32 -> out[128,258,258]f32.

    scale = a^2 where the separable filter is f = [a, 3a, 3a, a].
    strips: list of strip heights summing to H (small edge strips shorten
            pipeline fill/drain).  v_dve_edge: for the first/last N strips
            compute V=3U on the DVE (ts 4x) instead of ACT, shortening the
            per-strip critical path at the pipeline ends.
    """
    import concourse.bass as bass
    import concourse.mybir as mybir
    from concourse.tile import TileContext

    f32 = mybir.dt.float32
    bf16 = mybir.dt.bfloat16
    Copy = mybir.ActivationFunctionType.Copy
    add = mybir.AluOpType.add
    if strips is None:
        strips = [1, 1, 2, 4] + [8] * 14 + [4, 2, 1, 1]
    assert sum(strips) == H
    nstrips = len(strips)
    hs_max = max(strips)
    m0s = list(np.cumsum([0] + strips[:-1]))
    zdt = bf16 if out_mode == "bf16_swdge" else f32

    nc = bass.Bass()
    x = nc.dram_tensor("x", [NIMG, H, W], f32, kind="ExternalInput")
    out = nc.dram_tensor("out", [NIMG, OH, OW], f32, kind="ExternalOutput")

    with TileContext(nc) as tc:
        with tc.tile_pool(name="p", bufs=2) as pool:
            in_eng = {"sync": nc.sync, "scalar": nc.scalar,
                      "tensor": nc.tensor}[in_ring]

            # persistent U [130 x 258] bf16; ghost rows 0 and 129 are zero.
            # All startup memsets go on the (otherwise idle) GpSimd queue so
            # they don't delay strip-0's DVE ops.
            U = pool.tile([NIMG, H + 2, OW], bf16, tag="U", bufs=1)
            nc.gpsimd.memset(U[:, 0:1, :], 0.0)
            nc.gpsimd.memset(U[:, H + 1:H + 2, :], 0.0)

            # input chunks (row0, nrows), boundaries aligned to strip
            # boundaries: tiny first chunks (sync ring, FIFO) let strip-0
            # compute start at once; the 1.5MB chunks alternate between the
            # sync and scalar HWDGE rings so two transfers stream in parallel
            # during the output ramp (one FIFO ring serializes them at
            # ~370 GB/s vs the ~430 wire).  Every big chunk gets its own
            # buffer (bufs=5): a buffer-reuse wait on the scalar ring would
            # stall the ACT queue behind it.
            in_chunks = [(0, 1), (1, 1), (2, 2), (4, 4), (8, 24), (32, 24),
                         (56, 24), (80, 24), (104, 24)]
            assert sum(n for _, n in in_chunks) == H
            xtiles = []
            nbig = 0
            for (r0, nr) in in_chunks:
                big = nr > 4
                xt = pool.tile([NIMG, 24 if big else 4, W], f32,
                               tag="xl" if big else "xs", bufs=5 if big else 4)
                eng = in_eng
                if big:
                    eng = nc.scalar if nbig % 2 else nc.sync
                    nbig += 1
                eng.dma_start(out=xt[:, 0:nr, :], in_=x[:, r0:r0 + nr, :])
                xtiles.append((r0, nr, xt))

            def x_view(m0, hs):
                for r0, nr, xt in xtiles:
                    if r0 <= m0 and m0 + hs <= r0 + nr:
                        return xt[:, m0 - r0:m0 - r0 + hs, :]
                raise AssertionError(f"strip [{m0},{m0+hs}) spans input chunks")

            # pre-allocate S/T strip buffers and zero their ghost cols once
            # (buffers rotate; ghosts are never overwritten afterwards)
            sbufs, tbufs = [], []
            for b in range(bufs_s):
                st = pool.tile([NIMG, hs_max, W + 2], bf16, tag="st", bufs=bufs_s)
                nc.gpsimd.memset(st[:, :, 0:1], 0.0)
                nc.gpsimd.memset(st[:, :, W + 1:W + 2], 0.0)
                sbufs.append(st)
                tt_ = pool.tile([NIMG, hs_max, W + 2], bf16, tag="tt", bufs=bufs_s)
                nc.gpsimd.memset(tt_[:, :, 0:1], 0.0)
                nc.gpsimd.memset(tt_[:, :, W + 1:W + 2], 0.0)
                tbufs.append(tt_)

            for s in range(nstrips):
                hs = strips[s]
                m0 = int(m0s[s])

                # S[r, j] = scale * X[r, j-1]  (j = 1..128; ghosts j=0,129)
                st = sbufs[s % bufs_s]
                nc.scalar.activation(st[:, 0:hs, 1:W + 1], x_view(m0, hs),
                                     Copy, scale=scale)
                # T = 3S (DVE ts 4x) over all 130 cols (ghosts stay 0)
                tt_ = tbufs[s % bufs_s]
                nc.vector.tensor_scalar_mul(tt_[:, 0:hs, :], st[:, 0:hs, :], 3.0)

                # W-pass -> U rows m0+1 .. m0+hs (u = r+1)
                # U[u, 2m]   = 3S[m] + S[m+1] = T[m] + S[m+1]   m = 0..128
                # U[u, 2m+1] = S[m] + 3S[m+1] = S[m] + T[m+1]
                nc.vector.tensor_tensor(
                    out=U[:, m0 + 1:m0 + hs + 1, 0:2 * W + 2:2],
                    in0=tt_[:, 0:hs, 0:W + 1], in1=st[:, 0:hs, 1:W + 2], op=add)
                nc.vector.tensor_tensor(
                    out=U[:, m0 + 1:m0 + hs + 1, 1:2 * W + 2:2],
                    in0=st[:, 0:hs, 0:W + 1], in1=tt_[:, 0:hs, 1:W + 2], op=add)

                # V = 3U rows m0 .. m0+n_m  (ghost U rows give V=0).  ACT in
                # the steady state; DVE ts (4x) on edge strips to shorten the
                # pipeline-fill/drain critical path.
                n_m = hs + (1 if s == nstrips - 1 else 0)
                vt_full = pool.tile([NIMG, hs_max + 2, OW], bf16, tag="vt",
                                    bufs=bufs_v)
                vt = vt_full[:, 0:n_m + 1, :]
                if s < v_dve_edge or s >= nstrips - v_dve_edge:
                    nc.vector.tensor_scalar_mul(vt, U[:, m0:m0 + n_m + 1, :], 3.0)
                else:
                    nc.scalar.activation(vt, U[:, m0:m0 + n_m + 1, :],
                                         Copy, scale=3.0)

                # H-pass: Z rows 2m0 .. 2(m0+n_m)-1
                # Z[2m] = V[m] + U[m+1];  Z[2m+1] = U[m] + V[m+1]
                nch = split_last if s == nstrips - 1 else 1
                bounds = [n_m * c // nch for c in range(nch + 1)]
                for c in range(nch):
                    j0, j1 = bounds[c], bounds[c + 1]
                    nj = j1 - j0
                    if nj == 0:
                        continue
                    zt_full = pool.tile([NIMG, 2 * hs_max + 2, OW], zdt, tag="zt",
                                        bufs=bufs_z)
                    zt = zt_full[:, 0:2 * nj, :]
                    nc.vector.tensor_tensor(
                        out=zt[:, 0:2 * nj:2, :],
                        in0=vt[:, j0:j1, :],
                        in1=U[:, m0 + j0 + 1:m0 + j1 + 1, :], op=add)
                    nc.vector.tensor_tensor(
                        out=zt[:, 1:2 * nj:2, :],
                        in0=U[:, m0 + j0:m0 + j1, :],
                        in1=vt[:, j0 + 1:j1 + 1, :], op=add)
                    oeng = nc.gpsimd if out_mode == "bf16_swdge" else nc.sync
                    oeng.dma_start(
                        out=out[:, 2 * (m0 + j0):2 * (m0 + j1), :],
                        in_=zt[:, :, :])

    _legalize_waits(nc, mybir)
    return nc


def _taps_from_w(w):
    """Recover separable 4-tap filter f (filt = outer(f, f)) from w[0, 0];
    return a^2 where f = [a, 3a, 3a, a]."""
    filt = np.asarray(w, dtype=np.float32)[0, 0]
    j = int(np.argmax(np.abs(np.diag(filt))))
    f = filt[:, j] / np.float32(np.sqrt(filt[j, j]))
    assert np.allclose(np.outer(f, f), filt, atol=1e-5), "filter not separable"
    assert abs(f[0] - f[3]) < 1e-6 and abs(f[1] - f[2]) < 1e-6, "not symmetric"
    assert abs(f[1] - 3 * f[0]) < 1e-5, "not the 3:1 bilinear tap"
    return float(f[0]) * float(f[0])


BEST_CFG = dict()


def _get_nc(scale, **cfg):
    cfg = {**BEST_CFG, **cfg}
    key = (round(scale, 8), tuple(sorted(cfg.items())))
    if key not in _CACHE:
        _CACHE[key] = _build_bass(scale, **cfg)
    return _CACHE[key]


def run_sharded(x, w, cfg=None, **run_kwargs):
    from concourse.bass_utils import run_bass_kernel_spmd

    x = np.ascontiguousarray(np.asarray(x, dtype=np.float32))
    scale = _taps_from_w(w)
    nc = _get_nc(scale, **(cfg or {}))

    in_maps = []
    for k in range(NCORES):
        xk = np.ascontiguousarray(
            x[:, k * CPC:(k + 1) * CPC].reshape(NIMG, H, W))
        in_maps.append({"x": xk})

    res = run_bass_kernel_spmd(nc, in_maps, core_ids=list(range(NCORES)),
                               **run_kwargs)

    full = np.empty((N, C, OH, OW), dtype=np.float32)
    for k in range(NCORES):
        full[:, k * CPC:(k + 1) * CPC] = res.results[k]["out"].reshape(
            N, CPC, OH, OW)
    return full, res


def kernel(x, w):
    full, _ = run_sharded(x, w)
    return full
